# revision 38
# baseline (speedup 1.0000x reference)
"""DeepSets encoder kernel for 8 Trainium2 NeuronCores.

Strategy (all shapes hardcoded for the graded problem):
  - per-point MLP phi: Linear(16,256) -> LN -> ReLU -> Linear(256,256) -> LN
    -> ReLU -> Linear(256,128), then ragged segment mean + broadcast back.
  - LN mean-centering is folded into the weights on the host (exact).
  - LN rstd factors commute through ReLU/matmul into one per-point scale
      s = rsqrt(var2' + eps*var1 + eps^2)
    applied at the very end; var1 comes from a 17x17 Cholesky quadratic-form
    trick, var2' from squares of the centered layer-2 preactivation.
  - Data-parallel across 8 cores at segment granularity; each core gets a
    fully specialized static program (segment geometry baked in).
"""

import dataclasses
import numpy as np
import ml_dtypes

import concourse.bass as bass
import concourse.tile as tile
import concourse.mybir as mybir
from concourse import bacc

BF16 = ml_dtypes.bfloat16

AF = mybir.ActivationFunctionType
ALU = mybir.AluOpType
DT = mybir.dt

B = 2000
D_IN = 16
H = 256
D_OUT = 128
EPS = 1e-5
T = 512          # points per tile
G = 16           # tiles per stats group
SEGBLK = 32      # segments per psum accumulation block
NCORES = 8


# ----------------------------------------------------------------------------
# host-side planning
# ----------------------------------------------------------------------------

def _make_plans(counts):
    """Split segments into 8 contiguous shards with ~equal point counts."""
    n = counts.sum()
    starts = np.concatenate([[0], np.cumsum(counts)])
    plans = []
    s0 = 0
    for c in range(NCORES):
        target = (c + 1) * n / NCORES
        if c == NCORES - 1:
            s1 = len(counts)
        else:
            s1 = int(np.searchsorted(starts, target))
            s1 = max(s1, s0 + 1)
        plans.append(dict(s0=s0, s1=s1, p0=int(starts[s0]), p1=int(starts[s1])))
        s0 = s1
    return plans


@dataclasses.dataclass
class CoreProg:
    nc: object
    in_map: dict
    out_name: str
    p0: int
    p1: int


def _build_core(plan, z, consts):
    """Build one core's fully specialized program."""
    s0, s1, p0, p1 = plan["s0"], plan["s1"], plan["p0"], plan["p1"]
    counts = consts["counts"][s0:s1]
    npts = p1 - p0
    ntiles = (npts + T - 1) // T
    npad = ntiles * T
    nseg = len(counts)

    # local segment boundaries (within [0, npad)); fake pad segment at end
    bnd = np.concatenate([[0], np.cumsum(counts)]).astype(np.int64)

    # seg index for every local point (pad points -> -1)
    segidx = np.full(npad, -1, np.int64)
    for s in range(nseg):
        segidx[bnd[s]:bnd[s + 1]] = s

    # host-transposed padded z (+ ones row for the bias-augmented ext matmul)
    zt = np.zeros((17, npad), BF16)
    zt[:16, :npts] = z[p0:p1].T.astype(BF16)
    zt[16, :] = 1.0

    # S matrices per tile: [128, 4*32] fp16; chunk c cols [32c,32c+32) map
    # local chunk points to (seg % SEGBLK) of the chunk's primary block; a
    # chunk crossing a block boundary gets a secondary S via extra const.
    S_all = np.zeros((ntiles, 128, 128), np.float16)
    S_extra = {}      # (t, c) -> [128,32] fp16 for the secondary block
    chunk_blocks = {}  # (t, c) -> list of block ids present
    invcnt = 1.0 / counts.astype(np.float64)
    for t in range(ntiles):
        for c in range(4):
            base = t * T + c * 128
            segs_here = segidx[base:base + 128]
            blocks = sorted({int(s) // SEGBLK for s in np.unique(segs_here) if s >= 0})
            chunk_blocks[(t, c)] = blocks
            for p in range(128):
                s = segs_here[p]
                if s < 0:
                    continue
                blk = int(s) // SEGBLK
                col = int(s) % SEGBLK
                v = np.float16(invcnt[int(s)])
                if blk == blocks[0]:
                    S_all[t, p, 32 * c + col] = v
                else:
                    if (t, c) not in S_extra:
                        S_extra[(t, c)] = np.zeros((128, 32), np.float16)
                    S_extra[(t, c)][p, col] = v

    nblocks = (nseg + SEGBLK - 1) // SEGBLK
    # last tile index contributing to each block
    blk_last_tile = [0] * nblocks
    for (t, c), blocks in chunk_blocks.items():
        for b in blocks:
            blk_last_tile[b] = max(blk_last_tile[b], t)

    # batch S tiles 4-at-a-time: [ng4, 128, 4*128]
    ng4 = (ntiles + 3) // 4
    S4_all = np.zeros((ng4, 128, 512), np.float16)
    for t in range(ntiles):
        S4_all[t // 4, :, 128 * (t % 4):128 * (t % 4) + 128] = S_all[t]

    nc = bacc.Bacc("TRN2", target_bir_lowering=False, debug=False, num_devices=1)

    # ---- dram tensors ----
    d = {}
    def din(name, arr, dt_):
        d[name] = arr
        return nc.dram_tensor(name, list(arr.shape), dt_, kind="ExternalInput")

    zt_d = din("zt", zt, DT.bfloat16)
    w1a_d = din("w1a", consts["W1c"][:, :128].astype(BF16), DT.bfloat16)
    w1b_d = din("w1b", consts["W1c"][:, 128:].astype(BF16), DT.bfloat16)
    lw_d = din("lw", consts["Lw"].astype(BF16), DT.bfloat16)
    w2_d = din("w2", consts["W2cb"].astype(BF16), DT.bfloat16)  # [2,2,128,128] (kc, mh)
    w3_d = din("w3", consts["W3b"], DT.float16)            # [2,128,128]
    ec_d = din("ec", consts["ec"].astype(BF16), DT.bfloat16)    # [128, 47] ones col 15
    vb_d = din("vb", consts["vb"].astype(np.float16), DT.float16)            # [128, 47] diag blocks
    ones1_d = din("ones1", np.ones((1, 128), np.float16), DT.float16)
    sel_np = np.zeros((32, 4096), np.float16)
    for k_ in range(32):
        sel_np[k_, 128 * k_:128 * k_ + 128] = 1.0
    sel_d = din("sel", sel_np, DT.float16)
    eye_d = din("eye", np.eye(16, dtype=np.float32), DT.float32)
    b1s_d = din("b1s", consts["bias1"], DT.float32)        # [2,128,1] relu1 bias per half
    b2s_d = din("b2s", consts["bias2"], DT.float32)        # [2,128,1]
    b2c_d = din("b2c", consts["b2c2"], DT.float32)         # [2,128,1] square bias
    g1s_d = din("g1s", consts["g1s"], DT.float32)          # [2,128,1]
    g2s_d = din("g2s", consts["g2s"], DT.float32)
    S_d = din("S", S4_all, DT.float16)                     # [ng4,128,512]
    Sx_items = sorted(S_extra.items())
    if Sx_items:
        Sx_arr = np.stack([v for _, v in Sx_items])
    else:
        Sx_arr = np.zeros((1, 128, 32), np.float16)
    Sx_d = din("Sx", Sx_arr, DT.float16)
    Sx_idx = {k: i for i, (k, _) in enumerate(Sx_items)}

    out_d = nc.dram_tensor("out", [npts, D_OUT], DT.float32, kind="ExternalOutput")

    trivial = consts["trivial"]

    with tile.TileContext(nc) as tc:
        with (
            tc.tile_pool(name="wp", bufs=1) as wp,
            tc.tile_pool(name="zp", bufs=3) as zp,
            tc.tile_pool(name="ap", bufs=2) as apool,
            tc.tile_pool(name="sp", bufs=3) as spool,
            tc.tile_pool(name="a2p", bufs=G + 6) as a2p,
            tc.tile_pool(name="fp", bufs=2) as fpool,
            tc.tile_pool(name="mp", bufs=2) as mpool,
            tc.tile_pool(name="Spl", bufs=3) as Spl,
            tc.tile_pool(name="ph1", bufs=1, space="PSUM") as ph1,
            tc.tile_pool(name="px2", bufs=1, space="PSUM") as px2,
            tc.tile_pool(name="pex", bufs=1, space="PSUM") as pex,
            tc.tile_pool(name="pst", bufs=1, space="PSUM") as pst,
            tc.tile_pool(name="pph", bufs=2, space="PSUM") as pph,
        ):
            # ---- persistent weight tiles ----
            w1a = wp.tile([16, 128], DT.bfloat16, tag="w1a")
            nc.sync.dma_start(w1a[:], w1a_d[:, :])
            w1b = wp.tile([16, 128], DT.bfloat16, tag="w1b")
            nc.sync.dma_start(w1b[:], w1b_d[:, :])
            lw = wp.tile([17, 224], DT.bfloat16, tag="lw")
            nc.sync.dma_start(lw[:], lw_d[:, :])
            w2 = wp.tile([128, 512], DT.bfloat16, tag="w2")
            for kc_ in range(2):
                for mh_ in range(2):
                    nc.sync.dma_start(w2[:, (2 * kc_ + mh_) * 128:(2 * kc_ + mh_) * 128 + 128],
                                      w2_d[kc_, mh_, :, :])
            w3 = wp.tile([128, 256], DT.float16, tag="w3")
            for kc_ in range(2):
                nc.sync.dma_start(w3[:, 128 * kc_:128 * kc_ + 128], w3_d[kc_, :, :])
            ec = wp.tile([128, 47], DT.bfloat16, tag="ec")
            nc.sync.dma_start(ec[:], ec_d[:, :])
            vb = wp.tile([128, 47], DT.float16, tag="vb")
            nc.sync.dma_start(vb[:], vb_d[:, :])
            ones1 = wp.tile([1, 128], DT.float16, tag="ones1")
            nc.sync.dma_start(ones1[:], ones1_d[:, :])
            sel = wp.tile([32, 4096], DT.float16, tag="sel")
            nc.sync.dma_start(sel[:], sel_d[:, :])
            eye16 = wp.tile([16, 16], DT.float32, tag="eye16")
            nc.sync.dma_start(eye16[:], eye_d[:, :])
            b1s = wp.tile([128, 2], DT.float32, tag="b1s")
            for mh_ in range(2):
                nc.sync.dma_start(b1s[:, mh_:mh_ + 1], b1s_d[mh_, :, :])
            b2s = wp.tile([128, 2], DT.float32, tag="b2s")
            for mh_ in range(2):
                nc.sync.dma_start(b2s[:, mh_:mh_ + 1], b2s_d[mh_, :, :])
            b2c = wp.tile([128, 2], DT.float32, tag="b2c")
            for mh_ in range(2):
                nc.sync.dma_start(b2c[:, mh_:mh_ + 1], b2c_d[mh_, :, :])
            g1s = wp.tile([128, 2], DT.float32, tag="g1s")
            for mh_ in range(2):
                nc.sync.dma_start(g1s[:, mh_:mh_ + 1], g1s_d[mh_, :, :])
            g2s = wp.tile([128, 2], DT.float32, tag="g2s")
            for mh_ in range(2):
                nc.sync.dma_start(g2s[:, mh_:mh_ + 1], g2s_d[mh_, :, :])

            a2_tiles = {}
            sq2_tiles = {}
            sqe_tiles = {}
            seg_started = [False, False]

            def emit_phase_a_core(t):
                j = t % 4
                if j == 0:
                    ncols = min(4 * T, npad - t * T)
                    zt4 = zp.tile([17, 2048], DT.bfloat16, tag="zt4")
                    nc.gpsimd.dma_start(zt4[:, 0:ncols],
                                        zt_d[:, t * T:t * T + ncols])
                    emit_phase_a_core.zt4 = zt4
                zt_t = emit_phase_a_core.zt4[:, j * T:(j + 1) * T]
                h1 = ph1.tile([128, 1024], DT.float32, tag="h1")
                nc.tensor.matmul(h1[:, 0:512], w1a[:], zt_t[0:16, :], start=True, stop=True)
                nc.tensor.matmul(h1[:, 512:1024], w1b[:], zt_t[0:16, :], start=True, stop=True)
                if j == 0:
                    emit_phase_a_core.extps = pex.tile([128, 512], DT.float32, tag="ext")
                extps = emit_phase_a_core.extps
                nc.tensor.matmul(extps[:], lw[:, 96 - 32 * j:224 - 32 * j], zt_t[:, :],
                                 start=(j == 0), stop=True)
                a1 = apool.tile([128, 1024], DT.bfloat16, tag="a1")
                if trivial:
                    # split across DVE and ACT to balance the elementwise load
                    nc.vector.tensor_scalar(a1[:, 0:512], h1[:, 0:512],
                                            0.0, None, ALU.max)
                    nc.scalar.activation(a1[:, 512:1024], h1[:, 512:1024], AF.Relu)
                else:
                    for mh in range(2):
                        nc.scalar.activation(a1[:, 512 * mh:512 * mh + 512],
                                             h1[:, 512 * mh:512 * mh + 512], AF.Relu,
                                             bias=b1s[:, mh:mh + 1], scale=g1s[:, mh:mh + 1])
                x2 = px2.tile([128, 1024], DT.float32, tag="x2")
                for mh in range(2):
                    for kc in range(2):
                        nc.tensor.matmul(x2[:, 512 * mh:512 * mh + 512],
                                         w2[:, (2 * kc + mh) * 128:(2 * kc + mh) * 128 + 128],
                                         a1[:, 512 * kc:512 * kc + 512],
                                         start=(kc == 0), stop=(kc == 1))
                sq2 = spool.tile([128, 1024], DT.bfloat16, tag="sq2")
                if trivial:
                    nc.scalar.activation(sq2[:], x2[:], AF.Square)
                else:
                    for mh in range(2):
                        nc.scalar.activation(sq2[:, 512 * mh:512 * mh + 512],
                                             x2[:, 512 * mh:512 * mh + 512], AF.Square,
                                             bias=b2c[:, mh:mh + 1])
                sq2_tiles[t] = sq2
                if j == 3 or t == ntiles - 1:
                    # square the quad's ext bands now so the pex bank frees up
                    sqe = spool.tile([128, 512], DT.float16, tag="sqe")
                    nc.scalar.activation(sqe[:], extps[:], AF.Square)
                    sqe_tiles[t // 4] = sqe
                a2 = a2p.tile([128, 1024], DT.float16, tag="a2")
                if trivial:
                    nc.vector.tensor_scalar(a2[:], x2[:], 0.0, None, ALU.max)
                else:
                    for mh in range(2):
                        nc.vector.tensor_scalar(a2[:, 512 * mh:512 * mh + 512],
                                                x2[:, 512 * mh:512 * mh + 512],
                                                g2s[:, mh:mh + 1], 0.0, ALU.mult, ALU.max)
                a2_tiles[t] = a2

            def emit_var2(t):
                # deferred one step so sq2(t) is ready when PE gets here
                r = t % G
                sq2 = sq2_tiles.pop(t)
                # the two mh halves go to different col groups so they run
                # concurrently on the PE array
                nc.tensor.matmul(misc[0:32, :], ec[:, 15 - r:47 - r], sq2[:, 0:512],
                                 start=(r == 0), stop=True)
                nc.tensor.matmul(misc[32:64, :], ec[:, 15 - r:47 - r], sq2[:, 512:1024],
                                 start=(r == 0), stop=True)
                if t % 4 == 3 or t == ntiles - 1:
                    r0 = 4 * (r // 4)
                    sqe = sqe_tiles.pop(t // 4)
                    nc.tensor.matmul(misc[64:96, :], vb[:, 15 - r0:47 - r0], sqe[:],
                                     start=(r0 == 0), stop=True)

            def emit_sbatch(g, gtiles):
                u1 = fpool.tile([16, 512], DT.float32, tag="u1")
                nc.vector.tensor_scalar(u1[:], misc[64:80, :], EPS / H, EPS * EPS,
                                        ALU.mult, ALU.add)
                v2s = fpool.tile([16, 512], DT.float32, tag="v2s")
                nc.vector.scalar_tensor_tensor(v2s[:], misc[0:16, :], 1.0 / H, u1[:],
                                               ALU.mult, ALU.add)
                tval = fpool.tile([16, 512], DT.float32, tag="tval")
                nc.vector.scalar_tensor_tensor(tval[:], misc[32:48, :], 1.0 / H, v2s[:],
                                               ALU.mult, ALU.add)
                sraw = fpool.tile([16, 512], DT.float32, tag="sraw")
                nc.scalar.activation(sraw[:], tval[:], AF.Abs_reciprocal_sqrt)
                stp = pph.tile([128, 64], DT.float32, tag="phi")
                for b in range(4):
                    nc.tensor.transpose(stp[:, 16 * b:16 * b + 16],
                                        sraw[:, 128 * b:128 * b + 128], eye16[:])
                sT_t = fpool.tile([128, 64], DT.float32, tag="sT")
                nc.vector.tensor_copy(sT_t[:], stp[:])
                return sT_t

            phs_tiles = {}
            S4_tiles = {}
            sx_tiles = {}

            def emit_phi_phs(t, sT_t):
                r = t % G
                # prefetch S matrices one step ahead of their seg matmuls
                if t % 4 == 0:
                    S4 = Spl.tile([128, 512], DT.float16, tag="S4")
                    nc.gpsimd.dma_start(S4[:], S_d[t // 4, :, :])
                    S4_tiles[t // 4] = S4
                for c in range(4):
                    if (t, c) in Sx_idx and len(chunk_blocks.get((t, c), [])) > 1:
                        sx = Spl.tile([128, 32], DT.float16, tag="Sx")
                        nc.gpsimd.dma_start(sx[:], Sx_d[Sx_idx[(t, c)], :, :])
                        sx_tiles[(t, c)] = sx
                a2 = a2_tiles.pop(t)
                phi = pph.tile([128, 512], DT.float32, tag="phi")
                for c in range(4):
                    for kc in range(2):
                        nc.tensor.matmul(phi[:, 128 * c:128 * c + 128],
                                         a2[:, 512 * kc + 128 * c:512 * kc + 128 * c + 128],
                                         w3[:, 128 * kc:128 * kc + 128],
                                         start=(kc == 0), stop=(kc == 1))
                phs = mpool.tile([128, 512], DT.float16, tag="phs")
                for c in range(4):
                    col = 16 * c + r
                    if c % 2 == 0:
                        nc.scalar.activation(phs[:, 128 * c:128 * c + 128],
                                             phi[:, 128 * c:128 * c + 128], AF.Copy,
                                             scale=sT_t[:, col:col + 1])
                    else:
                        nc.vector.tensor_scalar(phs[:, 128 * c:128 * c + 128],
                                                phi[:, 128 * c:128 * c + 128],
                                                sT_t[:, col:col + 1], None, ALU.mult)
                phs_tiles[t] = phs

            def emit_seg(t):
                phs = phs_tiles.pop(t)
                if t % 4 == 3 or t == ntiles - 1:
                    S4_tiles.pop(t // 4 - 1, None)
                S_t = S4_tiles[t // 4][:, 128 * (t % 4):128 * (t % 4) + 128]
                for c in range(4):
                    blocks = chunk_blocks.get((t, c), [])
                    for bi, blk in enumerate(blocks):
                        half = blk % 2
                        if bi == 0:
                            lhs = S_t[:, 32 * c:32 * c + 32]
                        else:
                            lhs = sx_tiles.pop((t, c))[:]
                        st = not seg_started[half]
                        nc.tensor.matmul(misc[96:128, 128 * half:128 * half + 128],
                                         lhs, phs[:, 128 * c:128 * c + 128],
                                         start=st, stop=True,
                                         tile_position=(0, 96))
                        seg_started[half] = True

            means_tiles = {}

            def emit_means(blk):
                # means for block blk are complete in misc[96:128] half (blk%2);
                # copy them out two steps before the broadcast matmuls so the
                # PE never waits on the DVE queue for them
                half = blk % 2
                means = mpool.tile([32, 128], DT.float16, tag="means")
                nc.vector.tensor_copy(means[:], misc[96:128, 128 * half:128 * half + 128])
                seg_started[half] = False
                means_tiles[blk] = means

            def emit_block_out(blk):
                lo = blk * SEGBLK
                hi = min(nseg, lo + SEGBLK)
                cnt_here = hi - lo
                means = means_tiles.pop(blk)
                for q in range(0, cnt_here, 4):
                    ob = pph.tile([128, 512], DT.float32, tag="phi")
                    for kk in range(q, min(q + 4, cnt_here)):
                        # one-hot select row kk of means, broadcast across the
                        # 128 output partitions
                        nc.tensor.matmul(ob[:, 128 * (kk - q):128 * (kk - q) + 128],
                                         sel[:, 128 * kk:128 * kk + 128], means[:, :],
                                         start=True, stop=True)
                    osb = mpool.tile([128, 512], DT.float32, tag="osb")
                    if (q // 4) % 2 == 0:
                        nc.scalar.activation(osb[:], ob[:], AF.Copy)
                    else:
                        nc.vector.tensor_copy(osb[:], ob[:])
                    for k in range(q, min(q + 4, cnt_here)):
                        s_ = lo + k
                        start_row = int(bnd[s_])
                        cnt = int(counts[s_])
                        kk = k - q
                        nfull = cnt // 128
                        rem = cnt % 128
                        if nfull:
                            src = osb[:, 128 * kk:128 * kk + 128]
                            src = dataclasses.replace(
                                src, ap=[list(src.ap[0]), [0, nfull], list(src.ap[1])])
                            dst = out_d[start_row:start_row + 128 * nfull, :]
                            dst = dataclasses.replace(
                                dst, ap=[[128, 128], [128 * 128, nfull], [1, 128]])
                            nc.sync.dma_start(dst, src)
                        if rem:
                            nc.sync.dma_start(
                                out_d[start_row + 128 * nfull:start_row + cnt, :],
                                osb[0:rem, 128 * kk:128 * kk + 128])

            # ---- main emission: flat software pipeline ----
            # misc psum bank regions:
            #   rows  0:32  var2 accumulation (restart per group)
            #   rows 32:64  var1 accumulation (restart per quad)
            #   rows 96:128 segment sums, [32, 2*128] for even/odd blocks
            # Per step s the PE stream is:
            #   [phase_a(s), var2(s-1), phi(s-LAG), seg(s-LAG-1)]
            # so every matmul's elementwise inputs were produced >=1 full
            # step earlier and the PE never waits on same-tile ACT/DVE work.
            misc = pst.tile([128, 512], DT.float32, tag="misc")
            LAG = 19
            sT_map = {}
            done_blocks = 0
            means_done = 0
            for s in range(ntiles + LAG + 2):
                if s < ntiles:
                    emit_phase_a_core(s)
                tv = s - 2
                if 0 <= tv < ntiles:
                    emit_var2(tv)
                    if tv % G == G - 1 or tv == ntiles - 1:
                        sT_map[tv // G] = emit_sbatch(tv // G, None)
                tb = s - LAG
                if 0 <= tb < ntiles:
                    emit_phi_phs(tb, sT_map[tb // G])
                ts_ = s - LAG - 1
                if 0 <= ts_ < ntiles:
                    while (means_done < nblocks
                           and blk_last_tile[means_done] <= ts_ - 1):
                        emit_means(means_done)
                        means_done += 1
                    while (done_blocks < nblocks
                           and blk_last_tile[done_blocks] <= ts_ - 3):
                        emit_block_out(done_blocks)
                        done_blocks += 1
                    emit_seg(ts_)
            while means_done < nblocks:
                emit_means(means_done)
                means_done += 1
            while done_blocks < nblocks:
                emit_block_out(done_blocks)
                done_blocks += 1

    nc.compile()
    return CoreProg(nc=nc, in_map=d, out_name="out", p0=p0, p1=p1)


# ----------------------------------------------------------------------------
# host folding of weights
# ----------------------------------------------------------------------------

def _fold(inputs):
    W1 = np.asarray(inputs["W1"], np.float64)
    b1 = np.asarray(inputs["b1"], np.float64)
    g1 = np.asarray(inputs["g1"], np.float64)
    be1 = np.asarray(inputs["be1"], np.float64)
    W2 = np.asarray(inputs["W2"], np.float64)
    b2 = np.asarray(inputs["b2"], np.float64)
    g2 = np.asarray(inputs["g2"], np.float64)
    be2 = np.asarray(inputs["be2"], np.float64)
    W3 = np.asarray(inputs["W3"], np.float64)
    b3 = np.asarray(inputs["b3"], np.float64)

    # centered first layer
    W1c = W1 - W1.mean(axis=1, keepdims=True)
    b1c = b1 - b1.mean()
    # var1 quadratic form (bias-augmented)
    A = np.concatenate([W1c, b1c[None, :]], axis=0)      # [17, 256]
    M1 = A @ A.T
    Lc = np.linalg.cholesky(M1 + 1e-12 * np.eye(17))
    Lw = np.zeros((17, 224), np.float32)
    Lw[:, 96:113] = Lc.astype(np.float32)

    W2c = W2 - W2.mean(axis=1, keepdims=True)
    b2c = b2 - b2.mean()

    # blocks
    W2cb = np.zeros((2, 2, 128, 128), np.float32)        # [kc, mh]
    for kc in range(2):
        for mh in range(2):
            W2cb[kc, mh] = W2c[128 * kc:128 * kc + 128, 128 * mh:128 * mh + 128]
    W3b = np.zeros((2, 128, 128), np.float16)
    for kc in range(2):
        W3b[kc] = W3[128 * kc:128 * kc + 128, :].astype(np.float16)

    ec = np.zeros((128, 47), np.float32)
    ec[:, 15] = 1.0
    vb = np.zeros((128, 47), np.float32)
    for i in range(4):
        vb[32 * i:32 * i + 17, 15 + i] = 1.0

    trivial = (np.all(g1 == 1) and np.all(be1 == 0) and np.all(b1 == 0)
               and np.all(g2 == 1) and np.all(be2 == 0) and np.all(b2 == 0))

    bias1 = np.zeros((2, 128, 1), np.float32)
    bias2 = np.zeros((2, 128, 1), np.float32)
    b2c2 = np.zeros((2, 128, 1), np.float32)
    g1s = np.zeros((2, 128, 1), np.float32)
    g2s = np.zeros((2, 128, 1), np.float32)
    for mh in range(2):
        bias1[mh, :, 0] = (g1 * b1c + be1)[128 * mh:128 * mh + 128]
        bias2[mh, :, 0] = (g2 * b2c + be2)[128 * mh:128 * mh + 128]
        b2c2[mh, :, 0] = b2c[128 * mh:128 * mh + 128]
        g1s[mh, :, 0] = g1[128 * mh:128 * mh + 128]
        g2s[mh, :, 0] = g2[128 * mh:128 * mh + 128]

    return dict(
        W1c=W1c.astype(np.float32), Lw=Lw, W2cb=W2cb, W3b=W3b,
        ec=ec, vb=vb, bias1=bias1, bias2=bias2, b2c2=b2c2, g1s=g1s, g2s=g2s,
        trivial=trivial, b3=np.asarray(b3, np.float32),
    )


# ----------------------------------------------------------------------------
# execution: per-device async dispatch of 8 specialized programs
# ----------------------------------------------------------------------------

def _run_programs(progs):
    import jax
    from concourse import bass2jax

    bass2jax.install_neuronx_cc_hook()
    devices = jax.devices()
    futures = []
    for i, prog in enumerate(progs):
        nc = prog.nc
        in_names, out_names, out_avals, zero_outs = [], [], [], []
        for alloc in nc.m.functions[0].allocations:
            if not isinstance(alloc, mybir.MemoryLocationSet):
                continue
            name = alloc.memorylocations[0].name
            if alloc.kind == "ExternalInput":
                in_names.append(name)
            elif alloc.kind == "ExternalOutput":
                out_names.append(name)
                shape = tuple(alloc.tensor_shape)
                dtype = mybir.dt.np(alloc.dtype)
                out_avals.append(jax.core.ShapedArray(shape, dtype))
                zero_outs.append(np.zeros(shape, dtype))
        n_params = len(in_names)
        all_names = in_names + out_names

        def body(*args, nc=nc, out_avals=tuple(out_avals),
                 all_names=tuple(all_names), out_names=tuple(out_names)):
            outs = bass2jax._bass_exec_p.bind(
                *args, out_avals=out_avals, in_names=all_names,
                out_names=out_names, lowering_input_output_aliases=(),
                sim_require_finite=False, sim_require_nnan=False, nc=nc)
            return tuple(outs)

        donate = tuple(range(n_params, n_params + len(out_names)))
        jitted = jax.jit(body, donate_argnums=donate, keep_unused=True)
        dev = devices[i % len(devices)]
        pid_name = nc.partition_id_tensor.name if nc.partition_id_tensor else None
        in_map = dict(prog.in_map)
        if pid_name is not None and pid_name not in in_map:
            in_map[pid_name] = np.array([[i]], np.uint32)
        args = [jax.device_put(np.ascontiguousarray(in_map[n]), dev)
                for n in in_names]
        args += [jax.device_put(z, dev) for z in zero_outs]
        futures.append((jitted(*args), out_names))
    results = []
    for outs, out_names in futures:
        results.append({n: np.asarray(o) for n, o in zip(out_names, outs)})
    return results


_PROG_CACHE = {}


def build_programs(inputs):
    counts = np.asarray(inputs["num_points"]).astype(np.int64)
    key = counts.tobytes()
    consts = _fold(inputs)
    consts["counts"] = counts
    plans = _make_plans(counts)
    z = np.asarray(inputs["z_t"], np.float32)
    progs = [_build_core(p, z, consts) for p in plans]
    return progs, consts


def kernel(**inputs):
    progs, consts = build_programs(inputs)
    results = _run_programs(progs)
    out = np.empty((sum(p.p1 - p.p0 for p in progs), D_OUT), np.float32)
    for prog, res in zip(progs, results):
        out[prog.p0:prog.p1] = res[prog.out_name]
    b3 = consts["b3"]
    if np.any(b3):
        out += b3[None, :]
    return out



# revision 39
# speedup vs baseline: 1.1100x; 1.1100x over previous
"""DeepSets encoder kernel for 8 Trainium2 NeuronCores.

Strategy (all shapes hardcoded for the graded problem):
  - per-point MLP phi: Linear(16,256) -> LN -> ReLU -> Linear(256,256) -> LN
    -> ReLU -> Linear(256,128), then ragged segment mean + broadcast back.
  - LN mean-centering is folded into the weights on the host (exact).
  - LN rstd factors commute through ReLU/matmul into one per-point scale
      s = rsqrt(var2' + eps*var1 + eps^2)
    applied at the very end; var1 comes from a 17x17 Cholesky quadratic-form
    trick, var2' from squares of the centered layer-2 preactivation.
  - Data-parallel across 8 cores at segment granularity; each core gets a
    fully specialized static program (segment geometry baked in).
"""

import dataclasses
import numpy as np
import ml_dtypes

import concourse.bass as bass
import concourse.tile as tile
import concourse.mybir as mybir
from concourse import bacc

BF16 = ml_dtypes.bfloat16

AF = mybir.ActivationFunctionType
ALU = mybir.AluOpType
DT = mybir.dt

B = 2000
D_IN = 16
H = 256
D_OUT = 128
EPS = 1e-5
T = 512          # points per tile
G = 16           # tiles per stats group
SEGBLK = 32      # segments per psum accumulation block
NCORES = 8


# ----------------------------------------------------------------------------
# host-side planning
# ----------------------------------------------------------------------------

def _make_plans(counts):
    """Split segments into 8 contiguous shards with ~equal point counts."""
    n = counts.sum()
    starts = np.concatenate([[0], np.cumsum(counts)])
    plans = []
    s0 = 0
    for c in range(NCORES):
        target = (c + 1) * n / NCORES
        if c == NCORES - 1:
            s1 = len(counts)
        else:
            s1 = int(np.searchsorted(starts, target))
            s1 = max(s1, s0 + 1)
        plans.append(dict(s0=s0, s1=s1, p0=int(starts[s0]), p1=int(starts[s1])))
        s0 = s1
    return plans


@dataclasses.dataclass
class CoreProg:
    nc: object
    in_map: dict
    out_name: str
    p0: int
    p1: int


def _build_core(plan, z, consts):
    """Build one core's fully specialized program."""
    s0, s1, p0, p1 = plan["s0"], plan["s1"], plan["p0"], plan["p1"]
    counts = consts["counts"][s0:s1]
    npts = p1 - p0
    ntiles = (npts + T - 1) // T
    npad = ntiles * T
    nseg = len(counts)

    # local segment boundaries (within [0, npad)); fake pad segment at end
    bnd = np.concatenate([[0], np.cumsum(counts)]).astype(np.int64)

    # seg index for every local point (pad points -> -1)
    segidx = np.full(npad, -1, np.int64)
    for s in range(nseg):
        segidx[bnd[s]:bnd[s + 1]] = s

    # host-transposed padded z (+ ones row for the bias-augmented ext matmul)
    zt = np.zeros((17, npad), BF16)
    zt[:16, :npts] = z[p0:p1].T.astype(BF16)
    zt[16, :] = 1.0

    # S matrices per tile: [128, 4*32] fp16; chunk c cols [32c,32c+32) map
    # local chunk points to (seg % SEGBLK) of the chunk's primary block; a
    # chunk crossing a block boundary gets a secondary S via extra const.
    S_all = np.zeros((ntiles, 128, 128), np.float16)
    S_extra = {}      # (t, c) -> [128,32] fp16 for the secondary block
    chunk_blocks = {}  # (t, c) -> list of block ids present
    invcnt = 1.0 / counts.astype(np.float64)
    for t in range(ntiles):
        for c in range(4):
            base = t * T + c * 128
            segs_here = segidx[base:base + 128]
            blocks = sorted({int(s) // SEGBLK for s in np.unique(segs_here) if s >= 0})
            chunk_blocks[(t, c)] = blocks
            for p in range(128):
                s = segs_here[p]
                if s < 0:
                    continue
                blk = int(s) // SEGBLK
                col = int(s) % SEGBLK
                v = np.float16(invcnt[int(s)])
                if blk == blocks[0]:
                    S_all[t, p, 32 * c + col] = v
                else:
                    if (t, c) not in S_extra:
                        S_extra[(t, c)] = np.zeros((128, 32), np.float16)
                    S_extra[(t, c)][p, col] = v

    nblocks = (nseg + SEGBLK - 1) // SEGBLK
    # last tile index contributing to each block
    blk_last_tile = [0] * nblocks
    for (t, c), blocks in chunk_blocks.items():
        for b in blocks:
            blk_last_tile[b] = max(blk_last_tile[b], t)

    # batch S tiles 4-at-a-time: [ng4, 128, 4*128]
    ng4 = (ntiles + 3) // 4
    S4_all = np.zeros((ng4, 128, 512), np.float16)
    for t in range(ntiles):
        S4_all[t // 4, :, 128 * (t % 4):128 * (t % 4) + 128] = S_all[t]

    nc = bacc.Bacc("TRN2", target_bir_lowering=False, debug=False, num_devices=1)

    # ---- dram tensors ----
    d = {}
    def din(name, arr, dt_):
        d[name] = arr
        return nc.dram_tensor(name, list(arr.shape), dt_, kind="ExternalInput")

    zt_d = din("zt", zt, DT.bfloat16)
    w1a_d = din("w1a", consts["W1c"][:, :128].astype(BF16), DT.bfloat16)
    w1b_d = din("w1b", consts["W1c"][:, 128:].astype(BF16), DT.bfloat16)
    lw_d = din("lw", consts["Lw"].astype(BF16), DT.bfloat16)
    w2_d = din("w2", consts["W2cb"].astype(BF16), DT.bfloat16)  # [2,2,128,128] (kc, mh)
    w3_d = din("w3", consts["W3b"], DT.float16)            # [2,128,128]
    ec_d = din("ec", consts["ec"].astype(BF16), DT.bfloat16)    # [128, 47] ones col 15
    vb_d = din("vb", consts["vb"].astype(np.float16), DT.float16)            # [128, 47] diag blocks
    ones1_d = din("ones1", np.ones((1, 128), np.float16), DT.float16)
    sel_np = np.zeros((32, 4096), np.float16)
    for k_ in range(32):
        sel_np[k_, 128 * k_:128 * k_ + 128] = 1.0
    sel_d = din("sel", sel_np, DT.float16)
    eye_d = din("eye", np.eye(16, dtype=np.float32), DT.float32)
    b1s_d = din("b1s", consts["bias1"], DT.float32)        # [2,128,1] relu1 bias per half
    b2s_d = din("b2s", consts["bias2"], DT.float32)        # [2,128,1]
    b2c_d = din("b2c", consts["b2c2"], DT.float32)         # [2,128,1] square bias
    g1s_d = din("g1s", consts["g1s"], DT.float32)          # [2,128,1]
    g2s_d = din("g2s", consts["g2s"], DT.float32)
    S_d = din("S", S4_all, DT.float16)                     # [ng4,128,512]
    Sx_items = sorted(S_extra.items())
    if Sx_items:
        Sx_arr = np.stack([v for _, v in Sx_items])
    else:
        Sx_arr = np.zeros((1, 128, 32), np.float16)
    Sx_d = din("Sx", Sx_arr, DT.float16)
    Sx_idx = {k: i for i, (k, _) in enumerate(Sx_items)}

    out_d = nc.dram_tensor("out", [npts, D_OUT], DT.float32, kind="ExternalOutput")

    trivial = consts["trivial"]

    with tile.TileContext(nc) as tc:
        with (
            tc.tile_pool(name="wp", bufs=1) as wp,
            tc.tile_pool(name="zp", bufs=3) as zp,
            tc.tile_pool(name="ap", bufs=2) as apool,
            tc.tile_pool(name="sp", bufs=2) as spool,
            tc.tile_pool(name="a2p", bufs=G + 6) as a2p,
            tc.tile_pool(name="fp", bufs=2) as fpool,
            tc.tile_pool(name="mp", bufs=2) as mpool,
            tc.tile_pool(name="Spl", bufs=3) as Spl,
            tc.tile_pool(name="ph1", bufs=1, space="PSUM") as ph1,
            tc.tile_pool(name="px2", bufs=1, space="PSUM") as px2,
            tc.tile_pool(name="pex", bufs=1, space="PSUM") as pex,
            tc.tile_pool(name="pst", bufs=1, space="PSUM") as pst,
            tc.tile_pool(name="pph", bufs=2, space="PSUM") as pph,
        ):
            # ---- persistent weight tiles ----
            w1a = wp.tile([16, 128], DT.bfloat16, tag="w1a")
            nc.sync.dma_start(w1a[:], w1a_d[:, :])
            w1b = wp.tile([16, 128], DT.bfloat16, tag="w1b")
            nc.sync.dma_start(w1b[:], w1b_d[:, :])
            lw = wp.tile([17, 224], DT.bfloat16, tag="lw")
            nc.sync.dma_start(lw[:], lw_d[:, :])
            w2 = wp.tile([128, 512], DT.bfloat16, tag="w2")
            for kc_ in range(2):
                for mh_ in range(2):
                    nc.sync.dma_start(w2[:, (2 * kc_ + mh_) * 128:(2 * kc_ + mh_) * 128 + 128],
                                      w2_d[kc_, mh_, :, :])
            w3 = wp.tile([128, 256], DT.float16, tag="w3")
            for kc_ in range(2):
                nc.sync.dma_start(w3[:, 128 * kc_:128 * kc_ + 128], w3_d[kc_, :, :])
            ec = wp.tile([128, 47], DT.bfloat16, tag="ec")
            nc.sync.dma_start(ec[:], ec_d[:, :])
            vb = wp.tile([128, 47], DT.float16, tag="vb")
            nc.sync.dma_start(vb[:], vb_d[:, :])
            ones1 = wp.tile([1, 128], DT.float16, tag="ones1")
            nc.sync.dma_start(ones1[:], ones1_d[:, :])
            sel = wp.tile([32, 4096], DT.float16, tag="sel")
            nc.sync.dma_start(sel[:], sel_d[:, :])
            eye16 = wp.tile([16, 16], DT.float32, tag="eye16")
            nc.sync.dma_start(eye16[:], eye_d[:, :])
            b1s = wp.tile([128, 2], DT.float32, tag="b1s")
            for mh_ in range(2):
                nc.sync.dma_start(b1s[:, mh_:mh_ + 1], b1s_d[mh_, :, :])
            b2s = wp.tile([128, 2], DT.float32, tag="b2s")
            for mh_ in range(2):
                nc.sync.dma_start(b2s[:, mh_:mh_ + 1], b2s_d[mh_, :, :])
            b2c = wp.tile([128, 2], DT.float32, tag="b2c")
            for mh_ in range(2):
                nc.sync.dma_start(b2c[:, mh_:mh_ + 1], b2c_d[mh_, :, :])
            g1s = wp.tile([128, 2], DT.float32, tag="g1s")
            for mh_ in range(2):
                nc.sync.dma_start(g1s[:, mh_:mh_ + 1], g1s_d[mh_, :, :])
            g2s = wp.tile([128, 2], DT.float32, tag="g2s")
            for mh_ in range(2):
                nc.sync.dma_start(g2s[:, mh_:mh_ + 1], g2s_d[mh_, :, :])

            a2_tiles = {}
            sq2_tiles = {}
            sqe_tiles = {}
            seg_started = [False, False]

            def emit_phase_a_core(t):
                j = t % 4
                if j == 0:
                    ncols = min(4 * T, npad - t * T)
                    zt4 = zp.tile([17, 2048], DT.bfloat16, tag="zt4")
                    nc.gpsimd.dma_start(zt4[:, 0:ncols],
                                        zt_d[:, t * T:t * T + ncols])
                    emit_phase_a_core.zt4 = zt4
                zt_t = emit_phase_a_core.zt4[:, j * T:(j + 1) * T]
                h1 = ph1.tile([128, 1024], DT.float32, tag="h1")
                nc.tensor.matmul(h1[:, 0:512], w1a[:], zt_t[0:16, :], start=True, stop=True)
                nc.tensor.matmul(h1[:, 512:1024], w1b[:], zt_t[0:16, :], start=True, stop=True)
                if j == 0:
                    emit_phase_a_core.extps = pex.tile([128, 512], DT.float32, tag="ext")
                extps = emit_phase_a_core.extps
                nc.tensor.matmul(extps[:], lw[:, 96 - 32 * j:224 - 32 * j], zt_t[:, :],
                                 start=(j == 0), stop=True)
                a1 = apool.tile([128, 1024], DT.bfloat16, tag="a1")
                if trivial:
                    # split across DVE and ACT to balance the elementwise load
                    nc.vector.tensor_scalar(a1[:, 0:512], h1[:, 0:512],
                                            0.0, None, ALU.max)
                    nc.scalar.activation(a1[:, 512:1024], h1[:, 512:1024], AF.Relu)
                else:
                    for mh in range(2):
                        nc.scalar.activation(a1[:, 512 * mh:512 * mh + 512],
                                             h1[:, 512 * mh:512 * mh + 512], AF.Relu,
                                             bias=b1s[:, mh:mh + 1], scale=g1s[:, mh:mh + 1])
                x2 = px2.tile([128, 1024], DT.float32, tag="x2")
                for mh in range(2):
                    for kc in range(2):
                        nc.tensor.matmul(x2[:, 512 * mh:512 * mh + 512],
                                         w2[:, (2 * kc + mh) * 128:(2 * kc + mh) * 128 + 128],
                                         a1[:, 512 * kc:512 * kc + 512],
                                         start=(kc == 0), stop=(kc == 1))
                sq2 = spool.tile([128, 1024], DT.bfloat16, tag="sq2")
                if trivial:
                    nc.scalar.activation(sq2[:], x2[:], AF.Square)
                else:
                    for mh in range(2):
                        nc.scalar.activation(sq2[:, 512 * mh:512 * mh + 512],
                                             x2[:, 512 * mh:512 * mh + 512], AF.Square,
                                             bias=b2c[:, mh:mh + 1])
                sq2_tiles[t] = sq2
                if j == 3 or t == ntiles - 1:
                    # square the quad's ext bands now so the pex bank frees up
                    sqe = spool.tile([128, 512], DT.float16, tag="sqe")
                    nc.scalar.activation(sqe[:], extps[:], AF.Square)
                    sqe_tiles[t // 4] = sqe
                a2 = a2p.tile([128, 1024], DT.float16, tag="a2")
                if trivial:
                    nc.vector.tensor_scalar(a2[:], x2[:], 0.0, None, ALU.max)
                else:
                    for mh in range(2):
                        nc.vector.tensor_scalar(a2[:, 512 * mh:512 * mh + 512],
                                                x2[:, 512 * mh:512 * mh + 512],
                                                g2s[:, mh:mh + 1], 0.0, ALU.mult, ALU.max)
                a2_tiles[t] = a2

            def emit_var2(t):
                # deferred one step so sq2(t) is ready when PE gets here
                r = t % G
                sq2 = sq2_tiles.pop(t)
                # the two mh halves go to different col groups so they run
                # concurrently on the PE array
                nc.tensor.matmul(misc[0:32, :], ec[:, 15 - r:47 - r], sq2[:, 0:512],
                                 start=(r == 0), stop=True)
                nc.tensor.matmul(misc[32:64, :], ec[:, 15 - r:47 - r], sq2[:, 512:1024],
                                 start=(r == 0), stop=True)
                if t % 4 == 3 or t == ntiles - 1:
                    r0 = 4 * (r // 4)
                    sqe = sqe_tiles.pop(t // 4)
                    nc.tensor.matmul(misc[64:96, :], vb[:, 15 - r0:47 - r0], sqe[:],
                                     start=(r0 == 0), stop=True)

            def emit_sbatch(g, gtiles):
                u1 = fpool.tile([16, 512], DT.float32, tag="u1")
                nc.vector.tensor_scalar(u1[:], misc[64:80, :], EPS / H, EPS * EPS,
                                        ALU.mult, ALU.add)
                v2s = fpool.tile([16, 512], DT.float32, tag="v2s")
                nc.vector.scalar_tensor_tensor(v2s[:], misc[0:16, :], 1.0 / H, u1[:],
                                               ALU.mult, ALU.add)
                tval = fpool.tile([16, 512], DT.float32, tag="tval")
                nc.vector.scalar_tensor_tensor(tval[:], misc[32:48, :], 1.0 / H, v2s[:],
                                               ALU.mult, ALU.add)
                sraw = fpool.tile([16, 512], DT.float32, tag="sraw")
                nc.scalar.activation(sraw[:], tval[:], AF.Abs_reciprocal_sqrt)
                stp = pph.tile([128, 64], DT.float32, tag="phi")
                for b in range(4):
                    nc.tensor.transpose(stp[:, 16 * b:16 * b + 16],
                                        sraw[:, 128 * b:128 * b + 128], eye16[:])
                sT_t = fpool.tile([128, 64], DT.float32, tag="sT")
                nc.vector.tensor_copy(sT_t[:], stp[:])
                return sT_t

            phs_tiles = {}
            S4_tiles = {}
            sx_tiles = {}

            def emit_phi_phs(t, sT_t):
                r = t % G
                # prefetch S matrices one step ahead of their seg matmuls
                if t % 4 == 0:
                    S4 = Spl.tile([128, 512], DT.float16, tag="S4")
                    nc.gpsimd.dma_start(S4[:], S_d[t // 4, :, :])
                    S4_tiles[t // 4] = S4
                for c in range(4):
                    if (t, c) in Sx_idx and len(chunk_blocks.get((t, c), [])) > 1:
                        sx = Spl.tile([128, 32], DT.float16, tag="Sx")
                        nc.gpsimd.dma_start(sx[:], Sx_d[Sx_idx[(t, c)], :, :])
                        sx_tiles[(t, c)] = sx
                a2 = a2_tiles.pop(t)
                phi = pph.tile([128, 512], DT.float32, tag="phi")
                for c in range(4):
                    for kc in range(2):
                        nc.tensor.matmul(phi[:, 128 * c:128 * c + 128],
                                         a2[:, 512 * kc + 128 * c:512 * kc + 128 * c + 128],
                                         w3[:, 128 * kc:128 * kc + 128],
                                         start=(kc == 0), stop=(kc == 1))
                phs = mpool.tile([128, 512], DT.float16, tag="phs")
                for c in range(4):
                    col = 16 * c + r
                    if c % 2 == 0:
                        nc.scalar.activation(phs[:, 128 * c:128 * c + 128],
                                             phi[:, 128 * c:128 * c + 128], AF.Copy,
                                             scale=sT_t[:, col:col + 1])
                    else:
                        nc.vector.tensor_scalar(phs[:, 128 * c:128 * c + 128],
                                                phi[:, 128 * c:128 * c + 128],
                                                sT_t[:, col:col + 1], None, ALU.mult)
                phs_tiles[t] = phs

            def emit_seg(t):
                phs = phs_tiles.pop(t)
                if t % 4 == 3 or t == ntiles - 1:
                    S4_tiles.pop(t // 4 - 1, None)
                S_t = S4_tiles[t // 4][:, 128 * (t % 4):128 * (t % 4) + 128]
                for c in range(4):
                    blocks = chunk_blocks.get((t, c), [])
                    for bi, blk in enumerate(blocks):
                        half = blk % 2
                        if bi == 0:
                            lhs = S_t[:, 32 * c:32 * c + 32]
                        else:
                            lhs = sx_tiles.pop((t, c))[:]
                        st = not seg_started[half]
                        nc.tensor.matmul(misc[96:128, 128 * half:128 * half + 128],
                                         lhs, phs[:, 128 * c:128 * c + 128],
                                         start=st, stop=True,
                                         tile_position=(0, 96))
                        seg_started[half] = True

            means_tiles = {}

            def emit_means(blk):
                # means for block blk are complete in misc[96:128] half (blk%2);
                # copy them out two steps before the broadcast matmuls so the
                # PE never waits on the DVE queue for them
                half = blk % 2
                means = mpool.tile([32, 128], DT.float16, tag="means")
                nc.vector.tensor_copy(means[:], misc[96:128, 128 * half:128 * half + 128])
                seg_started[half] = False
                means_tiles[blk] = means

            def emit_block_out(blk):
                lo = blk * SEGBLK
                hi = min(nseg, lo + SEGBLK)
                cnt_here = hi - lo
                means = means_tiles.pop(blk)
                for q in range(0, cnt_here, 4):
                    ob = pph.tile([128, 512], DT.float32, tag="phi")
                    for kk in range(q, min(q + 4, cnt_here)):
                        # one-hot select row kk of means, broadcast across the
                        # 128 output partitions
                        nc.tensor.matmul(ob[:, 128 * (kk - q):128 * (kk - q) + 128],
                                         sel[:, 128 * kk:128 * kk + 128], means[:, :],
                                         start=True, stop=True)
                    osb = mpool.tile([128, 512], DT.float32, tag="osb")
                    if (q // 4) % 2 == 0:
                        nc.scalar.activation(osb[:], ob[:], AF.Copy)
                    else:
                        nc.vector.tensor_copy(osb[:], ob[:])
                    for k in range(q, min(q + 4, cnt_here)):
                        s_ = lo + k
                        start_row = int(bnd[s_])
                        cnt = int(counts[s_])
                        kk = k - q
                        nfull = cnt // 128
                        rem = cnt % 128
                        if nfull:
                            src = osb[:, 128 * kk:128 * kk + 128]
                            src = dataclasses.replace(
                                src, ap=[list(src.ap[0]), [0, nfull], list(src.ap[1])])
                            dst = out_d[start_row:start_row + 128 * nfull, :]
                            dst = dataclasses.replace(
                                dst, ap=[[128, 128], [128 * 128, nfull], [1, 128]])
                            nc.sync.dma_start(dst, src)
                        if rem:
                            nc.sync.dma_start(
                                out_d[start_row + 128 * nfull:start_row + cnt, :],
                                osb[0:rem, 128 * kk:128 * kk + 128])

            # ---- main emission: flat software pipeline ----
            # misc psum bank regions:
            #   rows  0:32  var2 accumulation (restart per group)
            #   rows 32:64  var1 accumulation (restart per quad)
            #   rows 96:128 segment sums, [32, 2*128] for even/odd blocks
            # Per step s the PE stream is:
            #   [phase_a(s), var2(s-1), phi(s-LAG), seg(s-LAG-1)]
            # so every matmul's elementwise inputs were produced >=1 full
            # step earlier and the PE never waits on same-tile ACT/DVE work.
            misc = pst.tile([128, 512], DT.float32, tag="misc")
            LAG = 19
            sT_map = {}
            done_blocks = 0
            means_done = 0
            for s in range(ntiles + LAG + 2):
                if s < ntiles:
                    emit_phase_a_core(s)
                tv = s - 1
                if 0 <= tv < ntiles:
                    emit_var2(tv)
                    if tv % G == G - 1 or tv == ntiles - 1:
                        sT_map[tv // G] = emit_sbatch(tv // G, None)
                tb = s - LAG
                if 0 <= tb < ntiles:
                    emit_phi_phs(tb, sT_map[tb // G])
                ts_ = s - LAG - 1
                if 0 <= ts_ < ntiles:
                    while (means_done < nblocks
                           and blk_last_tile[means_done] <= ts_ - 1):
                        emit_means(means_done)
                        means_done += 1
                    while (done_blocks < nblocks
                           and blk_last_tile[done_blocks] <= ts_ - 3):
                        emit_block_out(done_blocks)
                        done_blocks += 1
                    emit_seg(ts_)
            while means_done < nblocks:
                emit_means(means_done)
                means_done += 1
            while done_blocks < nblocks:
                emit_block_out(done_blocks)
                done_blocks += 1

    nc.compile()
    return CoreProg(nc=nc, in_map=d, out_name="out", p0=p0, p1=p1)


# ----------------------------------------------------------------------------
# host folding of weights
# ----------------------------------------------------------------------------

def _fold(inputs):
    W1 = np.asarray(inputs["W1"], np.float64)
    b1 = np.asarray(inputs["b1"], np.float64)
    g1 = np.asarray(inputs["g1"], np.float64)
    be1 = np.asarray(inputs["be1"], np.float64)
    W2 = np.asarray(inputs["W2"], np.float64)
    b2 = np.asarray(inputs["b2"], np.float64)
    g2 = np.asarray(inputs["g2"], np.float64)
    be2 = np.asarray(inputs["be2"], np.float64)
    W3 = np.asarray(inputs["W3"], np.float64)
    b3 = np.asarray(inputs["b3"], np.float64)

    # centered first layer
    W1c = W1 - W1.mean(axis=1, keepdims=True)
    b1c = b1 - b1.mean()
    # var1 quadratic form (bias-augmented)
    A = np.concatenate([W1c, b1c[None, :]], axis=0)      # [17, 256]
    M1 = A @ A.T
    Lc = np.linalg.cholesky(M1 + 1e-12 * np.eye(17))
    Lw = np.zeros((17, 224), np.float32)
    Lw[:, 96:113] = Lc.astype(np.float32)

    W2c = W2 - W2.mean(axis=1, keepdims=True)
    b2c = b2 - b2.mean()

    # blocks
    W2cb = np.zeros((2, 2, 128, 128), np.float32)        # [kc, mh]
    for kc in range(2):
        for mh in range(2):
            W2cb[kc, mh] = W2c[128 * kc:128 * kc + 128, 128 * mh:128 * mh + 128]
    W3b = np.zeros((2, 128, 128), np.float16)
    for kc in range(2):
        W3b[kc] = W3[128 * kc:128 * kc + 128, :].astype(np.float16)

    ec = np.zeros((128, 47), np.float32)
    ec[:, 15] = 1.0
    vb = np.zeros((128, 47), np.float32)
    for i in range(4):
        vb[32 * i:32 * i + 17, 15 + i] = 1.0

    trivial = (np.all(g1 == 1) and np.all(be1 == 0) and np.all(b1 == 0)
               and np.all(g2 == 1) and np.all(be2 == 0) and np.all(b2 == 0))

    bias1 = np.zeros((2, 128, 1), np.float32)
    bias2 = np.zeros((2, 128, 1), np.float32)
    b2c2 = np.zeros((2, 128, 1), np.float32)
    g1s = np.zeros((2, 128, 1), np.float32)
    g2s = np.zeros((2, 128, 1), np.float32)
    for mh in range(2):
        bias1[mh, :, 0] = (g1 * b1c + be1)[128 * mh:128 * mh + 128]
        bias2[mh, :, 0] = (g2 * b2c + be2)[128 * mh:128 * mh + 128]
        b2c2[mh, :, 0] = b2c[128 * mh:128 * mh + 128]
        g1s[mh, :, 0] = g1[128 * mh:128 * mh + 128]
        g2s[mh, :, 0] = g2[128 * mh:128 * mh + 128]

    return dict(
        W1c=W1c.astype(np.float32), Lw=Lw, W2cb=W2cb, W3b=W3b,
        ec=ec, vb=vb, bias1=bias1, bias2=bias2, b2c2=b2c2, g1s=g1s, g2s=g2s,
        trivial=trivial, b3=np.asarray(b3, np.float32),
    )


# ----------------------------------------------------------------------------
# execution: per-device async dispatch of 8 specialized programs
# ----------------------------------------------------------------------------

def _run_programs(progs):
    import jax
    from concourse import bass2jax

    bass2jax.install_neuronx_cc_hook()
    devices = jax.devices()
    futures = []
    for i, prog in enumerate(progs):
        nc = prog.nc
        in_names, out_names, out_avals, zero_outs = [], [], [], []
        for alloc in nc.m.functions[0].allocations:
            if not isinstance(alloc, mybir.MemoryLocationSet):
                continue
            name = alloc.memorylocations[0].name
            if alloc.kind == "ExternalInput":
                in_names.append(name)
            elif alloc.kind == "ExternalOutput":
                out_names.append(name)
                shape = tuple(alloc.tensor_shape)
                dtype = mybir.dt.np(alloc.dtype)
                out_avals.append(jax.core.ShapedArray(shape, dtype))
                zero_outs.append(np.zeros(shape, dtype))
        n_params = len(in_names)
        all_names = in_names + out_names

        def body(*args, nc=nc, out_avals=tuple(out_avals),
                 all_names=tuple(all_names), out_names=tuple(out_names)):
            outs = bass2jax._bass_exec_p.bind(
                *args, out_avals=out_avals, in_names=all_names,
                out_names=out_names, lowering_input_output_aliases=(),
                sim_require_finite=False, sim_require_nnan=False, nc=nc)
            return tuple(outs)

        donate = tuple(range(n_params, n_params + len(out_names)))
        jitted = jax.jit(body, donate_argnums=donate, keep_unused=True)
        dev = devices[i % len(devices)]
        pid_name = nc.partition_id_tensor.name if nc.partition_id_tensor else None
        in_map = dict(prog.in_map)
        if pid_name is not None and pid_name not in in_map:
            in_map[pid_name] = np.array([[i]], np.uint32)
        args = [jax.device_put(np.ascontiguousarray(in_map[n]), dev)
                for n in in_names]
        args += [jax.device_put(z, dev) for z in zero_outs]
        futures.append((jitted(*args), out_names))
    results = []
    for outs, out_names in futures:
        results.append({n: np.asarray(o) for n, o in zip(out_names, outs)})
    return results


_PROG_CACHE = {}


def build_programs(inputs):
    counts = np.asarray(inputs["num_points"]).astype(np.int64)
    key = counts.tobytes()
    consts = _fold(inputs)
    consts["counts"] = counts
    plans = _make_plans(counts)
    z = np.asarray(inputs["z_t"], np.float32)
    progs = [_build_core(p, z, consts) for p in plans]
    return progs, consts


def kernel(**inputs):
    progs, consts = build_programs(inputs)
    results = _run_programs(progs)
    out = np.empty((sum(p.p1 - p.p0 for p in progs), D_OUT), np.float32)
    for prog, res in zip(progs, results):
        out[prog.p0:prog.p1] = res[prog.out_name]
    b3 = consts["b3"]
    if np.any(b3):
        out += b3[None, :]
    return out

